# revision 41
# baseline (speedup 1.0000x reference)
"""RNN-T Joiner kernel for Trainium2 (Bass/Tile), SPMD over 8 NeuronCores.

Math: logits[b,t,u,v] = (enc@W_enc.T + b_enc + dec@W_dec.T + b_dec) @ W_out.T + b_out
    = A[b,t,v] + C[b,u,v]
where A = enc @ (W_out@W_enc).T  (no bias)
      C = (dec@W_dec.T + b_enc + b_dec) @ W_out.T + b_out

The (B,T,U,512)@(512,500) product in the reference (73.7 GFLOP) collapses by
linearity into two small matmuls plus a broadcast add, leaving the kernel
output-DMA bound (18 MB/core bf16 out, ~57 us at measured ~315 GB/s/core).

Design (per core, 2 batches), v3 — rebuilt from HW microbenchmarks:
- NO Pool/GPSIMD ops: on real HW each GPSIMD op costs ~2 us (the v1 one-hot
  'sel' pipeline measured 129 us standalone vs ~20 us modeled).
- C-row -> 128-partition broadcast via PE matmuls with a stride-0
  (broadcast) identity-column lhsT: crep[p,u,:] = eye[:,u](bcast 128).T @ C.
  Verified on HW; reads C's SBUF tile directly (no staging), any row index.
- C path reassociated: dec_projT = (dec @ W_dec.T).T via 16 tiny matmuls;
  b_enc+b_dec folded into the dec_projT PSUM->SBUF copies as a per-partition
  ACT bias; C = dec_projT.T @ W_out.T + ones.T@b_out (rank-1 inject).
- The broadcast add A[t,v]+C[u,v] runs as one wide bf16 DVE tensor_add per
  (batch, t-chunk, u-group) (free 15*500, 2x_1p mode) with A broadcast on a
  stride-0 free dim; first and last groups are sliced in 5-u chunks to cut
  pipeline head/tail latency.
- crep PSUM rows padded to 512 f32 (PSUM bank alignment), 2 rows per tile,
  3 rotating tiles; copies to bf16 group tiles run on ACT while DVE adds.
- Input DMAs ordered so the C chain (dec, W_dec.T, W_out) unblocks first;
  enc's first 128 columns are a separate DMA so A chunk 0 starts early.
- Output: bf16 logits (rel err ~5e-3), whole-group [tn,15,500] DMAs
  (15000 B contiguous DRAM rows) alternating the SP/ACT queues.

Sharding: data-parallel over batch B=16 -> 2 per core, no collectives.
Host-side work is layout only (slice / transpose / reshape / eye constant)
plus the final bf16->f32 upcast of the gathered output.
"""

import numpy as np

B, T, U, D, V = 16, 300, 30, 512, 500
NCORES = 8
BL = B // NCORES  # batches per core
P = 128
DC = D // P  # 4 contraction chunks

T_CHUNKS = [(0, 128), (128, 128), (256, 44)]
UH = 15  # u-group size (2 groups of 15)

_CACHE = {}


def _build_program(reps=1):
    from contextlib import ExitStack

    import concourse.bass as bass
    import concourse.tile as tile
    from concourse import bacc, mybir

    f32 = mybir.dt.float32
    f32r = mybir.dt.float32r
    bf16 = mybir.dt.bfloat16

    def r(ap):
        return ap.bitcast(f32r)

    nc = bacc.Bacc("TRN2", target_bir_lowering=False, debug=False)

    # aux packs the f32r matmul constants into one DMA -> one f32r tile
    # (fp32r operands must be real f32r tiles, not bitcast slices):
    # eye rows 0:60 cols 0:60, ones row 0 cols 64:192, bout row 0 cols 192:692.
    # bias_d packs b_enc/b_dec column-chunked: [128, 0:4]=benc, [:, 4:8]=bdec.
    AUXW = 704
    # weight/act inputs are host-rearranged so every DMA's src layout matches
    # its SBUF dst layout exactly (>=1.2 KB contiguous descriptors):
    #   enc_d [p, c, n] = enc.T[(c p), n], dec_d likewise
    #   wenc_d [dc, p, jc, d'] = W_enc[jc*128+p, dc*128+d']
    #   wdect_d [jc, p, c, j'] = W_dec.T[c*128+p, jc*128+j']
    #   wout_d [jc, p, v] = W_out.T[jc*128+p, v]
    enc_d = nc.dram_tensor("enc_d", [P, DC, BL * T], f32, kind="ExternalInput").ap()
    dec_d = nc.dram_tensor("dec_d", [P, DC, BL * U], f32, kind="ExternalInput").ap()
    wenc_d = nc.dram_tensor("wenc_d", [P, DC, DC, P], f32, kind="ExternalInput").ap()
    wdect_d = nc.dram_tensor("wdect_d", [P, DC, DC, P], f32, kind="ExternalInput").ap()
    wout_d = nc.dram_tensor("wout_d", [P, DC, V], f32, kind="ExternalInput").ap()
    aux_d = nc.dram_tensor("aux_d", [BL * U, AUXW], f32, kind="ExternalInput").ap()
    bias_d = nc.dram_tensor("bias_d", [P, 2 * DC], f32, kind="ExternalInput").ap()
    out = nc.dram_tensor("out", [BL, T, U, V], bf16, kind="ExternalOutput").ap()

    with tile.TileContext(nc) as tc:
        with ExitStack() as ctx:
            main = ctx.enter_context(tc.tile_pool(name="main", bufs=1))
            ps_a = ctx.enter_context(tc.tile_pool(name="ps_a", bufs=2, space="PSUM"))
            crep_ps = ctx.enter_context(
                tc.tile_pool(name="crep_ps", bufs=3, space="PSUM")
            )
            out_pool = ctx.enter_context(tc.tile_pool(name="outp", bufs=4))

            def body():
                # ---- persistent tiles ----
                enc_sb = main.tile([P, DC, BL * T], f32r, name="enc", tag="enc")
                dec_sb = main.tile([P, DC, BL * U], f32r, name="dec", tag="dec")
                wenc_sb = main.tile([P, DC, DC, P], f32r, name="wenc", tag="wenc")
                wdecT_sb = main.tile([P, DC, DC, P], f32r, name="wdecT", tag="wdecT")
                woutT_sb = main.tile([P, DC, V], f32r, name="woutT", tag="woutT")
                aux_sb = main.tile([BL * U, AUXW], f32r, name="aux", tag="aux")
                warm_sb = main.tile([1, 8], f32, name="warm", tag="warm")
                biasin_sb = main.tile([P, 2 * DC], f32, name="biasin", tag="biasin")
                bsum_sb = main.tile([P, DC], f32, name="bsum", tag="bsum")
                benc_sb = biasin_sb[:, 0:DC]
                bdec_sb = biasin_sb[:, DC : 2 * DC]
                eye_sb = aux_sb[0 : BL * U, 0 : BL * U]
                ones_sb = aux_sb[0:1, 64 : 64 + P]
                bout_sb = aux_sb[0:1, 192 : 192 + V]
                wceT_sb = [
                    main.tile([P, V], f32r, name=f"wceT{i}", tag=f"wceT{i}")
                    for i in range(DC)
                ]
                dpT_sb = [
                    main.tile([P, BL * U], f32r, name=f"dpT{i}", tag=f"dpT{i}")
                    for i in range(DC)
                ]
                a_sb = [
                    main.tile([P, V], bf16, name=f"a{i}", tag=f"a{i}")
                    for i in range(BL * len(T_CHUNKS))
                ]
                c_sb = main.tile([BL * U, V], f32r, name="c", tag="c")
                crepg = [
                    main.tile([P, UH, V], bf16, name=f"crepg{i}", tag=f"crepg{i}")
                    for i in range(4)
                ]

                # ---- input DMAs: C-path inputs first, big weights chunked so
                # the dpT/C/wce cascade starts as each 0.26 MB chunk lands ----
                # The ACT queue gets NO DMAs: dma_start occupies the issuing
                # sequencer for ~1us, which would stall the ACT copies that the
                # crep pipeline depends on. SP + DVE queues carry all DMAs.
                # 8 consolidated input DMAs (per-DMA issue overhead on HW is
                # ~1-3 us serialized on the queue, so fewer is faster); the
                # ACT queue takes three early ones (its first copy is at ~6us)
                # dummy Identity to trigger the ~1.5us ACT table load during
                # the input phase instead of before the first dpT copy
                nc.any.memset(warm_sb[:], 0.0)
                nc.scalar.activation(
                    warm_sb[:], warm_sb[:], mybir.ActivationFunctionType.Identity,
                    bias=0.0,
                )
                nc.sync.dma_start(aux_sb[:], r(aux_d[:]))
                nc.sync.dma_start(biasin_sb[:], bias_d[:])
                nc.sync.dma_start(dec_sb[:], r(dec_d[:]))
                nc.vector.tensor_add(bsum_sb[:], benc_sb[:], bdec_sb[:])
                for jc in range(DC):
                    nc.sync.dma_start(
                        wdecT_sb[:, jc, :, :], r(wdect_d[:, jc, :, :])
                    )
                    nc.scalar.dma_start(woutT_sb[:, jc, :], r(wout_d[:, jc, :]))
                nc.scalar.dma_start(wenc_sb[:], r(wenc_d[:]))
                nc.sync.dma_start(enc_sb[:, :, 0:T], r(enc_d[:, :, 0:T]))
                nc.sync.dma_start(enc_sb[:, :, T : BL * T], r(enc_d[:, :, T : BL * T]))

                # ---- dec_projT[jc][j, m] = sum_d W_dec[j,d] dec[m,d] + bsum[j]
                #      interleaved with C[m, v] = dec_projT.T @ W_out.T + b_out ----
                ps_c = crep_ps.tile([P, 2, 512], f32, name="cps", tag="cps")
                for jc in range(DC):
                    psd = crep_ps.tile([P, 2, 512], f32, name="cps", tag="cps")
                    ps = psd[:, 0, 0 : BL * U]
                    for dc in range(DC):
                        nc.tensor.matmul(
                            ps,
                            wdecT_sb[:, jc, dc, :],
                            dec_sb[:, dc, :],
                            start=(dc == 0),
                            stop=(dc == DC - 1),
                        )
                    nc.scalar.activation(
                        dpT_sb[jc][:],
                        ps,
                        mybir.ActivationFunctionType.Identity,
                        bias=bsum_sb[:, jc : jc + 1],
                    )
                    nc.tensor.matmul(
                        ps_c[0 : BL * U, 0, 0:V],
                        dpT_sb[jc][:],
                        woutT_sb[:, jc, :],
                        start=(jc == 0),
                        stop=False,
                    )
                nc.tensor.matmul(
                    ps_c[0 : BL * U, 0, 0:V],
                    ones_sb[0:1, 0 : BL * U],
                    bout_sb[:],
                    start=False,
                    stop=True,
                )
                nc.scalar.copy(c_sb[:], ps_c[0 : BL * U, 0, 0:V])

                # ---- Wce fusion (copies on DVE; ACT is busy with crep) ----
                def emit_wce_chunk(dc):
                    psw = crep_ps.tile([P, 2, 512], f32, name="cps", tag="cps")
                    ps = psw[:, 0, 0:V]
                    for jc in range(DC):
                        nc.tensor.matmul(
                            ps,
                            wenc_sb[:, dc, jc, :],
                            woutT_sb[:, jc, :],
                            start=(jc == 0),
                            stop=(jc == DC - 1),
                        )
                    nc.vector.tensor_copy(wceT_sb[dc][:], ps)

                def emit_a(bl, tci, interleaved=False, on_dve=False):
                    t0, tn = T_CHUNKS[tci]
                    n0 = bl * T + t0
                    ps = ps_a.tile([P, V], f32, name="psa", tag="a")
                    for dc in range(DC):
                        if interleaved:
                            emit_wce_chunk(dc)
                        nc.tensor.matmul(
                            ps[:tn, :],
                            enc_sb[:, dc, n0 : n0 + tn],
                            wceT_sb[dc][:],
                            start=(dc == 0),
                            stop=(dc == DC - 1),
                        )
                    dst = a_sb[bl * len(T_CHUNKS) + tci][:tn, :]
                    if on_dve:
                        nc.vector.tensor_copy(dst, ps[:tn, :])
                    else:
                        nc.scalar.copy(dst, ps[:tn, :])

                # ---- crep: broadcast C rows via identity-column matmuls ----
                def emit_crep(bl, g, lo=0, hi=UH):
                    i = lo
                    while i < hi:
                        row = bl * U + g * UH + i
                        n = min(2, hi - i)
                        cp = crep_ps.tile([P, 2, 512], f32, name="cps", tag="cps")
                        for k in range(n):
                            nc.tensor.matmul(
                                cp[:, k, 0:V],
                                eye_sb[:, row + k : row + k + 1].broadcast_to(
                                    [BL * U, P]
                                ),
                                c_sb[:],
                                start=True,
                                stop=True,
                            )
                        nc.scalar.copy(
                            crepg[bl * 2 + g][:, i : i + n, :], cp[:, :n, 0:V]
                        )
                        i += n

                # ---- output tiles: ot[t, u, v] = A[t,v] + crep[u][t,v] ----
                qi = [0]

                def emit_out_tile(bl, tci, g, sliced=False):
                    t0, tn = T_CHUNKS[tci]
                    a = a_sb[bl * len(T_CHUNKS) + tci]
                    cg = crepg[bl * 2 + g]
                    ot = out_pool.tile([P, UH, V], bf16, name="ot", tag="ot")
                    ab = a[:tn, :].unsqueeze(1)
                    slices = ((0, 5), (5, 5), (10, 5)) if sliced else ((0, UH),)
                    for s0, sn in slices:
                        nc.vector.tensor_add(
                            ot[:tn, s0 : s0 + sn, :],
                            ab.broadcast_to([tn, sn, V]),
                            cg[:tn, s0 : s0 + sn, :],
                        )
                        q = nc.sync
                        qi[0] += 1
                        q.dma_start(
                            out[bl, t0 : t0 + tn, g * UH + s0 : g * UH + s0 + sn, :],
                            ot[:tn, s0 : s0 + sn, :],
                        )

                emit_crep(0, 0, 0, 6)
                emit_a(0, 0, interleaved=True, on_dve=True)
                emit_crep(0, 0, 6, UH)
                emit_a(0, 1, on_dve=True)
                emit_out_tile(0, 0, 0, sliced=True)
                emit_a(0, 2)
                emit_crep(0, 1)
                emit_out_tile(0, 1, 0)
                emit_a(1, 0)
                emit_out_tile(0, 2, 0)
                emit_crep(1, 0)
                emit_a(1, 1)
                emit_a(1, 2)
                emit_crep(1, 1)
                emit_out_tile(0, 0, 1)
                emit_out_tile(0, 1, 1)
                emit_out_tile(0, 2, 1)
                emit_out_tile(1, 0, 0)
                emit_out_tile(1, 1, 0)
                emit_out_tile(1, 2, 0)
                emit_out_tile(1, 0, 1)
                emit_out_tile(1, 2, 1)
                emit_out_tile(1, 1, 1, sliced=True)

            if reps == 1:
                body()
            else:
                with tc.For_i(
                    0,
                    reps,
                    1,
                    hint_engines=(mybir.EngineType.PE, mybir.EngineType.Activation),
                ):
                    body()

    nc.compile()
    return nc


def _host_prep(inputs):
    """Per-core input maps. Layout-only host work (slice/transpose/reshape)."""
    enc = np.ascontiguousarray(inputs["encoder_out"], dtype=np.float32)
    dec = np.ascontiguousarray(inputs["decoder_out"], dtype=np.float32)
    # wenc_d[dc, p, jc, d'] = W_enc[jc*128+p, dc*128+d']
    wenc_d = np.ascontiguousarray(
        np.asarray(inputs["W_enc"], dtype=np.float32)
        .reshape(DC, P, DC, P)
        .transpose(1, 2, 0, 3)
    )
    # wdect_d[jc, p, c, j'] = W_dec.T[c*128+p, jc*128+j'] = W_dec[jc*128+j', c*128+p]
    wdect_d = np.ascontiguousarray(
        np.asarray(inputs["W_dec"], dtype=np.float32)
        .T.reshape(DC, P, DC, P)
        .transpose(1, 2, 0, 3)
    )
    # wout_d[jc, p, v] = W_out.T[jc*128+p, v] = W_out[v, jc*128+p]
    wout_d = np.ascontiguousarray(
        np.asarray(inputs["W_out"], dtype=np.float32).T.reshape(DC, P, V).transpose(1, 0, 2)
    )
    AUXW = 704
    aux = np.zeros((BL * U, AUXW), dtype=np.float32)
    aux[0 : BL * U, 0 : BL * U] = np.eye(BL * U, dtype=np.float32)
    aux[0:1, 64 : 64 + P] = 1.0
    aux[0:1, 192 : 192 + V] = inputs["b_out"].reshape(1, V)
    bias_np = np.concatenate(
        [
            inputs["b_enc"].reshape(DC, P).T,
            inputs["b_dec"].reshape(DC, P).T,
        ],
        axis=1,
    ).astype(np.float32)

    in_maps = []
    for c in range(NCORES):
        b0 = c * BL
        # enc_d[p, dc, n] = enc.T[dc*128+p, n]
        enc_c = np.ascontiguousarray(
            enc[b0 : b0 + BL].reshape(BL * T, DC, P).transpose(2, 1, 0)
        )
        dec_c = np.ascontiguousarray(
            dec[b0 : b0 + BL].reshape(BL * U, DC, P).transpose(2, 1, 0)
        )
        in_maps.append(
            {
                "enc_d": enc_c,
                "dec_d": dec_c,
                "wenc_d": wenc_d,
                "wdect_d": wdect_d,
                "wout_d": wout_d,
                "aux_d": aux,
                "bias_d": bias_np,
            }
        )
    return in_maps


def get_program(reps=1):
    key = f"nc{reps}"
    if key not in _CACHE:
        _CACHE[key] = _build_program(reps)
    return _CACHE[key]


def kernel(**inputs) -> np.ndarray:
    from concourse.bass_utils import run_bass_kernel_spmd

    nc = get_program()
    in_maps = _host_prep(inputs)
    res = run_bass_kernel_spmd(nc, in_maps, list(range(NCORES)))
    return np.concatenate(
        [np.asarray(r["out"]).astype(np.float32) for r in res.results], axis=0
    )


# revision 42
# speedup vs baseline: 1.0923x; 1.0923x over previous
"""RNN-T Joiner kernel for Trainium2 (Bass/Tile), SPMD over 8 NeuronCores.

Math: logits[b,t,u,v] = (enc@W_enc.T + b_enc + dec@W_dec.T + b_dec) @ W_out.T + b_out
    = A[b,t,v] + C[b,u,v]
where A = enc @ (W_out@W_enc).T  (no bias)
      C = (dec@W_dec.T + b_enc + b_dec) @ W_out.T + b_out

The (B,T,U,512)@(512,500) product in the reference (73.7 GFLOP) collapses by
linearity into two small matmuls plus a broadcast add, leaving the kernel
output-DMA bound (18 MB/core bf16 out, ~57 us at measured ~315 GB/s/core).

Design (per core, 2 batches), v3 — rebuilt from HW microbenchmarks:
- NO Pool/GPSIMD ops: on real HW each GPSIMD op costs ~2 us (the v1 one-hot
  'sel' pipeline measured 129 us standalone vs ~20 us modeled).
- C-row -> 128-partition broadcast via PE matmuls with a stride-0
  (broadcast) identity-column lhsT: crep[p,u,:] = eye[:,u](bcast 128).T @ C.
  Verified on HW; reads C's SBUF tile directly (no staging), any row index.
- C path reassociated: dec_projT = (dec @ W_dec.T).T via 16 tiny matmuls;
  b_enc+b_dec folded into the dec_projT PSUM->SBUF copies as a per-partition
  ACT bias; C = dec_projT.T @ W_out.T + ones.T@b_out (rank-1 inject).
- The broadcast add A[t,v]+C[u,v] runs as one wide bf16 DVE tensor_add per
  (batch, t-chunk, u-group) (free 15*500, 2x_1p mode) with A broadcast on a
  stride-0 free dim; first and last groups are sliced in 5-u chunks to cut
  pipeline head/tail latency.
- crep PSUM rows padded to 512 f32 (PSUM bank alignment), 2 rows per tile,
  3 rotating tiles; copies to bf16 group tiles run on ACT while DVE adds.
- Input DMAs ordered so the C chain (dec, W_dec.T, W_out) unblocks first;
  enc's first 128 columns are a separate DMA so A chunk 0 starts early.
- Output: bf16 logits (rel err ~5e-3), whole-group [tn,15,500] DMAs
  (15000 B contiguous DRAM rows) alternating the SP/ACT queues.

Sharding: data-parallel over batch B=16 -> 2 per core, no collectives.
Host-side work is layout only (slice / transpose / reshape / eye constant)
plus the final bf16->f32 upcast of the gathered output.
"""

import numpy as np

B, T, U, D, V = 16, 300, 30, 512, 500
NCORES = 8
BL = B // NCORES  # batches per core
P = 128
DC = D // P  # 4 contraction chunks

T_CHUNKS = [(0, 128), (128, 128), (256, 44)]
UH = 15  # u-group size (2 groups of 15)

_CACHE = {}


def _build_program(reps=1):
    from contextlib import ExitStack

    import concourse.bass as bass
    import concourse.tile as tile
    from concourse import bacc, mybir

    f32 = mybir.dt.float32
    f32r = mybir.dt.float32r
    bf16 = mybir.dt.bfloat16

    def r(ap):
        return ap.bitcast(f32r)

    nc = bacc.Bacc("TRN2", target_bir_lowering=False, debug=False)

    # aux packs the f32r matmul constants into one DMA -> one f32r tile
    # (fp32r operands must be real f32r tiles, not bitcast slices):
    # eye rows 0:60 cols 0:60, ones row 0 cols 64:192, bout row 0 cols 192:692.
    # bias_d packs b_enc/b_dec column-chunked: [128, 0:4]=benc, [:, 4:8]=bdec.
    AUXW = 704
    # weight/act inputs are host-rearranged so every DMA's src layout matches
    # its SBUF dst layout exactly (>=1.2 KB contiguous descriptors):
    #   enc_d [p, c, n] = enc.T[(c p), n], dec_d likewise
    #   wenc_d [dc, p, jc, d'] = W_enc[jc*128+p, dc*128+d']
    #   wdect_d [jc, p, c, j'] = W_dec.T[c*128+p, jc*128+j']
    #   wout_d [jc, p, v] = W_out.T[jc*128+p, v]
    enc_d = nc.dram_tensor("enc_d", [P, DC, BL * T], f32, kind="ExternalInput").ap()
    dec_d = nc.dram_tensor("dec_d", [P, DC, BL * U], f32, kind="ExternalInput").ap()
    wenc_d = nc.dram_tensor("wenc_d", [P, DC, DC, P], f32, kind="ExternalInput").ap()
    wdect_d = nc.dram_tensor("wdect_d", [P, DC, DC, P], f32, kind="ExternalInput").ap()
    wout_d = nc.dram_tensor("wout_d", [P, DC, V], f32, kind="ExternalInput").ap()
    aux_d = nc.dram_tensor("aux_d", [BL * U, AUXW], f32, kind="ExternalInput").ap()
    bias_d = nc.dram_tensor("bias_d", [P, 2 * DC], f32, kind="ExternalInput").ap()
    out = nc.dram_tensor("out", [BL, T, U, V], bf16, kind="ExternalOutput").ap()

    with tile.TileContext(nc) as tc:
        with ExitStack() as ctx:
            main = ctx.enter_context(tc.tile_pool(name="main", bufs=1))
            ps_a = ctx.enter_context(tc.tile_pool(name="ps_a", bufs=2, space="PSUM"))
            crep_ps = ctx.enter_context(
                tc.tile_pool(name="crep_ps", bufs=3, space="PSUM")
            )
            out_pool = ctx.enter_context(tc.tile_pool(name="outp", bufs=3))

            def body():
                # ---- persistent tiles ----
                enc_sb = main.tile([P, DC, BL * T], f32r, name="enc", tag="enc")
                dec_sb = main.tile([P, DC, BL * U], f32r, name="dec", tag="dec")
                wenc_sb = main.tile([P, DC, DC, P], f32r, name="wenc", tag="wenc")
                wdecT_sb = main.tile([P, DC, DC, P], f32r, name="wdecT", tag="wdecT")
                woutT_sb = main.tile([P, DC, V], f32r, name="woutT", tag="woutT")
                aux_sb = main.tile([BL * U, AUXW], f32r, name="aux", tag="aux")
                warm_sb = main.tile([1, 8], f32, name="warm", tag="warm")
                biasin_sb = main.tile([P, 2 * DC], f32, name="biasin", tag="biasin")
                bsum_sb = main.tile([P, DC], f32, name="bsum", tag="bsum")
                benc_sb = biasin_sb[:, 0:DC]
                bdec_sb = biasin_sb[:, DC : 2 * DC]
                eye_sb = aux_sb[0 : BL * U, 0 : BL * U]
                ones_sb = aux_sb[0:1, 64 : 64 + P]
                bout_sb = aux_sb[0:1, 192 : 192 + V]
                wceT_sb = [
                    main.tile([P, V], f32r, name=f"wceT{i}", tag=f"wceT{i}")
                    for i in range(DC)
                ]
                dpT_sb = [
                    main.tile([P, BL * U], f32r, name=f"dpT{i}", tag=f"dpT{i}")
                    for i in range(DC)
                ]
                a_sb = [
                    main.tile([P, V], bf16, name=f"a{i}", tag=f"a{i}")
                    for i in range(BL * len(T_CHUNKS))
                ]
                c_sb = main.tile([BL * U, V], f32r, name="c", tag="c")
                crepg = [
                    main.tile([P, UH, V], bf16, name=f"crepg{i}", tag=f"crepg{i}")
                    for i in range(4)
                ]

                # ---- input DMAs: C-path inputs first, big weights chunked so
                # the dpT/C/wce cascade starts as each 0.26 MB chunk lands ----
                # The ACT queue gets NO DMAs: dma_start occupies the issuing
                # sequencer for ~1us, which would stall the ACT copies that the
                # crep pipeline depends on. SP + DVE queues carry all DMAs.
                # 8 consolidated input DMAs (per-DMA issue overhead on HW is
                # ~1-3 us serialized on the queue, so fewer is faster); the
                # ACT queue takes three early ones (its first copy is at ~6us)
                # dummy Identity to trigger the ~1.5us ACT table load during
                # the input phase instead of before the first dpT copy
                nc.any.memset(warm_sb[:], 0.0)
                nc.scalar.activation(
                    warm_sb[:], warm_sb[:], mybir.ActivationFunctionType.Identity,
                    bias=0.0,
                )
                nc.sync.dma_start(aux_sb[:], r(aux_d[:]))
                nc.sync.dma_start(biasin_sb[:], bias_d[:])
                nc.sync.dma_start(dec_sb[:], r(dec_d[:]))
                nc.vector.tensor_add(bsum_sb[:], benc_sb[:], bdec_sb[:])
                for jc in range(DC):
                    nc.sync.dma_start(
                        wdecT_sb[:, jc, :, :], r(wdect_d[:, jc, :, :])
                    )
                    nc.scalar.dma_start(woutT_sb[:, jc, :], r(wout_d[:, jc, :]))
                nc.scalar.dma_start(wenc_sb[:], r(wenc_d[:]))
                nc.sync.dma_start(enc_sb[:, :, 0:T], r(enc_d[:, :, 0:T]))
                nc.sync.dma_start(enc_sb[:, :, T : BL * T], r(enc_d[:, :, T : BL * T]))

                # ---- dec_projT[jc][j, m] = sum_d W_dec[j,d] dec[m,d] + bsum[j]
                #      interleaved with C[m, v] = dec_projT.T @ W_out.T + b_out ----
                ps_c = crep_ps.tile([P, 2, 512], f32, name="cps", tag="cps")
                for jc in range(DC):
                    psd = crep_ps.tile([P, 2, 512], f32, name="cps", tag="cps")
                    ps = psd[:, 0, 0 : BL * U]
                    for dc in range(DC):
                        nc.tensor.matmul(
                            ps,
                            wdecT_sb[:, jc, dc, :],
                            dec_sb[:, dc, :],
                            start=(dc == 0),
                            stop=(dc == DC - 1),
                        )
                    nc.scalar.activation(
                        dpT_sb[jc][:],
                        ps,
                        mybir.ActivationFunctionType.Identity,
                        bias=bsum_sb[:, jc : jc + 1],
                    )
                    nc.tensor.matmul(
                        ps_c[0 : BL * U, 0, 0:V],
                        dpT_sb[jc][:],
                        woutT_sb[:, jc, :],
                        start=(jc == 0),
                        stop=False,
                    )
                nc.tensor.matmul(
                    ps_c[0 : BL * U, 0, 0:V],
                    ones_sb[0:1, 0 : BL * U],
                    bout_sb[:],
                    start=False,
                    stop=True,
                )
                nc.scalar.copy(c_sb[:], ps_c[0 : BL * U, 0, 0:V])

                # ---- Wce fusion (copies on DVE; ACT is busy with crep) ----
                def emit_wce_chunk(dc):
                    psw = crep_ps.tile([P, 2, 512], f32, name="cps", tag="cps")
                    ps = psw[:, 0, 0:V]
                    for jc in range(DC):
                        nc.tensor.matmul(
                            ps,
                            wenc_sb[:, dc, jc, :],
                            woutT_sb[:, jc, :],
                            start=(jc == 0),
                            stop=(jc == DC - 1),
                        )
                    nc.vector.tensor_copy(wceT_sb[dc][:], ps)

                def emit_a(bl, tci, interleaved=False, on_dve=False):
                    t0, tn = T_CHUNKS[tci]
                    n0 = bl * T + t0
                    ps = ps_a.tile([P, V], f32, name="psa", tag="a")
                    for dc in range(DC):
                        if interleaved:
                            emit_wce_chunk(dc)
                        nc.tensor.matmul(
                            ps[:tn, :],
                            enc_sb[:, dc, n0 : n0 + tn],
                            wceT_sb[dc][:],
                            start=(dc == 0),
                            stop=(dc == DC - 1),
                        )
                    dst = a_sb[bl * len(T_CHUNKS) + tci][:tn, :]
                    if on_dve:
                        nc.vector.tensor_copy(dst, ps[:tn, :])
                    else:
                        nc.scalar.copy(dst, ps[:tn, :])

                # ---- crep: broadcast C rows via identity-column matmuls ----
                def emit_crep(bl, g, lo=0, hi=UH):
                    i = lo
                    while i < hi:
                        row = bl * U + g * UH + i
                        n = min(2, hi - i)
                        cp = crep_ps.tile([P, 2, 512], f32, name="cps", tag="cps")
                        for k in range(n):
                            nc.tensor.matmul(
                                cp[:, k, 0:V],
                                eye_sb[:, row + k : row + k + 1].broadcast_to(
                                    [BL * U, P]
                                ),
                                c_sb[:],
                                start=True,
                                stop=True,
                            )
                        nc.scalar.copy(
                            crepg[bl * 2 + g][:, i : i + n, :], cp[:, :n, 0:V]
                        )
                        i += n

                # ---- output tiles: ot[t, u, v] = A[t,v] + crep[u][t,v] ----
                qi = [0]

                def emit_out_tile(bl, tci, g, sliced=False):
                    t0, tn = T_CHUNKS[tci]
                    a = a_sb[bl * len(T_CHUNKS) + tci]
                    cg = crepg[bl * 2 + g]
                    ot = out_pool.tile([P, UH, V], bf16, name="ot", tag="ot")
                    ab = a[:tn, :].unsqueeze(1)
                    slices = ((0, 5), (5, 5), (10, 5)) if sliced else ((0, UH),)
                    for s0, sn in slices:
                        nc.vector.tensor_add(
                            ot[:tn, s0 : s0 + sn, :],
                            ab.broadcast_to([tn, sn, V]),
                            cg[:tn, s0 : s0 + sn, :],
                        )
                        q = nc.sync
                        qi[0] += 1
                        q.dma_start(
                            out[bl, t0 : t0 + tn, g * UH + s0 : g * UH + s0 + sn, :],
                            ot[:tn, s0 : s0 + sn, :],
                        )

                emit_crep(0, 0, 0, 6)
                emit_a(0, 0, interleaved=True, on_dve=True)
                emit_crep(0, 0, 6, UH)
                emit_a(0, 1, on_dve=True)
                emit_out_tile(0, 0, 0, sliced=True)
                emit_a(0, 2)
                emit_crep(0, 1)
                emit_out_tile(0, 1, 0)
                emit_a(1, 0)
                emit_out_tile(0, 2, 0)
                emit_crep(1, 0)
                emit_a(1, 1)
                emit_a(1, 2)
                emit_out_tile(0, 0, 1)
                emit_crep(1, 1)
                emit_out_tile(0, 1, 1)
                emit_out_tile(0, 2, 1)
                emit_out_tile(1, 0, 0)
                emit_out_tile(1, 1, 0)
                emit_out_tile(1, 2, 0)
                emit_out_tile(1, 0, 1)
                emit_out_tile(1, 2, 1)
                emit_out_tile(1, 1, 1, sliced=True)

            if reps == 1:
                body()
            else:
                with tc.For_i(
                    0,
                    reps,
                    1,
                    hint_engines=(mybir.EngineType.PE, mybir.EngineType.Activation),
                ):
                    body()

    nc.compile()
    return nc


def _host_prep(inputs):
    """Per-core input maps. Layout-only host work (slice/transpose/reshape)."""
    enc = np.ascontiguousarray(inputs["encoder_out"], dtype=np.float32)
    dec = np.ascontiguousarray(inputs["decoder_out"], dtype=np.float32)
    # wenc_d[dc, p, jc, d'] = W_enc[jc*128+p, dc*128+d']
    wenc_d = np.ascontiguousarray(
        np.asarray(inputs["W_enc"], dtype=np.float32)
        .reshape(DC, P, DC, P)
        .transpose(1, 2, 0, 3)
    )
    # wdect_d[jc, p, c, j'] = W_dec.T[c*128+p, jc*128+j'] = W_dec[jc*128+j', c*128+p]
    wdect_d = np.ascontiguousarray(
        np.asarray(inputs["W_dec"], dtype=np.float32)
        .T.reshape(DC, P, DC, P)
        .transpose(1, 2, 0, 3)
    )
    # wout_d[jc, p, v] = W_out.T[jc*128+p, v] = W_out[v, jc*128+p]
    wout_d = np.ascontiguousarray(
        np.asarray(inputs["W_out"], dtype=np.float32).T.reshape(DC, P, V).transpose(1, 0, 2)
    )
    AUXW = 704
    aux = np.zeros((BL * U, AUXW), dtype=np.float32)
    aux[0 : BL * U, 0 : BL * U] = np.eye(BL * U, dtype=np.float32)
    aux[0:1, 64 : 64 + P] = 1.0
    aux[0:1, 192 : 192 + V] = inputs["b_out"].reshape(1, V)
    bias_np = np.concatenate(
        [
            inputs["b_enc"].reshape(DC, P).T,
            inputs["b_dec"].reshape(DC, P).T,
        ],
        axis=1,
    ).astype(np.float32)

    in_maps = []
    for c in range(NCORES):
        b0 = c * BL
        # enc_d[p, dc, n] = enc.T[dc*128+p, n]
        enc_c = np.ascontiguousarray(
            enc[b0 : b0 + BL].reshape(BL * T, DC, P).transpose(2, 1, 0)
        )
        dec_c = np.ascontiguousarray(
            dec[b0 : b0 + BL].reshape(BL * U, DC, P).transpose(2, 1, 0)
        )
        in_maps.append(
            {
                "enc_d": enc_c,
                "dec_d": dec_c,
                "wenc_d": wenc_d,
                "wdect_d": wdect_d,
                "wout_d": wout_d,
                "aux_d": aux,
                "bias_d": bias_np,
            }
        )
    return in_maps


def get_program(reps=1):
    key = f"nc{reps}"
    if key not in _CACHE:
        _CACHE[key] = _build_program(reps)
    return _CACHE[key]


def kernel(**inputs) -> np.ndarray:
    from concourse.bass_utils import run_bass_kernel_spmd

    nc = get_program()
    in_maps = _host_prep(inputs)
    res = run_bass_kernel_spmd(nc, in_maps, list(range(NCORES)))
    return np.concatenate(
        [np.asarray(r["out"]).astype(np.float32) for r in res.results], axis=0
    )


# revision 43
# speedup vs baseline: 1.1001x; 1.0071x over previous
"""RNN-T Joiner kernel for Trainium2 (Bass/Tile), SPMD over 8 NeuronCores.

Math: logits[b,t,u,v] = (enc@W_enc.T + b_enc + dec@W_dec.T + b_dec) @ W_out.T + b_out
    = A[b,t,v] + C[b,u,v]
where A = enc @ (W_out@W_enc).T  (no bias)
      C = (dec@W_dec.T + b_enc + b_dec) @ W_out.T + b_out

The (B,T,U,512)@(512,500) product in the reference (73.7 GFLOP) collapses by
linearity into two small matmuls plus a broadcast add, leaving the kernel
output-DMA / DVE-add bound (18 MB/core bf16 out at ~315 GB/s/core measured;
9M-element bf16 adds at DVE 2x_1p mode).

Design (per core, 2 batches) — iterated against real-HW NTFF traces:
- NO Pool/GPSIMD compute: on HW each GPSIMD op costs ~2 us (the one-hot
  'sel' broadcast pipeline of the original version measured 129 us
  standalone vs ~20 us modeled).
- C-row -> 128-partition broadcast via PE matmuls with a stride-0
  (broadcast) identity-column lhsT: crep[p,u,:] = eye[:,u](bcast 128).T @ C.
  HW-verified; reads C's SBUF tile directly, no staging, any row index.
  PSUM rows padded to 512 f32 (bank alignment), 2 rows per tile, 3 rotating
  tiles; PSUM->bf16 copies run on ACT while DVE does the adds.
- C path reassociated: dec_projT = (dec @ W_dec.T).T via 16 tiny matmuls
  (pipelined through the same 3-buf PSUM pool), b_enc+b_dec folded into the
  dec_projT copies as a per-partition ACT bias, C = dec_projT.T @ W_out.T
  + ones.T@b_out. Wce = W_out@W_enc fusion chunks rotate through the same
  pool; wce/a0/a1 PSUM->SBUF copies go to the (pre-add idle) DVE.
- The broadcast add A[t,v]+C[u,v] is one wide bf16 DVE tensor_add per
  (batch, t-chunk, u-group) (free 15*500, 2x_1p); on HW the 12 whole adds
  stream back-to-back at ~3.98 us. First and last groups are sliced in 5-u
  chunks to cut pipeline head/tail latency.
- Input DMAs: per-DMA end-to-end latency on HW is ~2.5 us serialized per
  queue, so small inputs are packed (aux: eye/ones/b_out; bias pair) and
  the three weight matrices load in host-pre-transposed, per-partition-
  contiguous layouts (~2 KB descriptors). W_dec.T/W_out.T load as per-jc
  chunks so the dec_projT/C cascade starts as each chunk lands; they are
  split across the SP and ACT queues. A dummy Identity op pre-triggers the
  ~1.5 us ACT table load during the input phase.
- Output: bf16 logits (rel err ~5e-3 vs f32 reference; host upcasts on
  gather), whole-group [tn,15,500] DMAs (15000 B contiguous DRAM rows),
  all on the SP queue so the ACT sequencer never stalls on DMA bookkeeping.

Sharding: data-parallel over batch B=16 -> 2 per core, no collectives.
Host-side work is layout only (slice / transpose / reshape / constants)
plus the final bf16->f32 upcast of the gathered output.
"""

import numpy as np

B, T, U, D, V = 16, 300, 30, 512, 500
NCORES = 8
BL = B // NCORES  # batches per core
P = 128
DC = D // P  # 4 contraction chunks

T_CHUNKS = [(0, 128), (128, 128), (256, 44)]
UH = 15  # u-group size (2 groups of 15)

_CACHE = {}


def _build_program(reps=1):
    from contextlib import ExitStack

    import concourse.bass as bass
    import concourse.tile as tile
    from concourse import bacc, mybir

    f32 = mybir.dt.float32
    f32r = mybir.dt.float32r
    bf16 = mybir.dt.bfloat16

    def r(ap):
        return ap.bitcast(f32r)

    nc = bacc.Bacc("TRN2", target_bir_lowering=False, debug=False)

    # aux packs the f32r matmul constants into one DMA -> one f32r tile
    # (fp32r operands must be real f32r tiles, not bitcast slices):
    # eye rows 0:60 cols 0:60, ones row 0 cols 64:192, bout row 0 cols 192:692.
    # bias_d packs b_enc/b_dec column-chunked: [128, 0:4]=benc, [:, 4:8]=bdec.
    AUXW = 704
    # weight/act inputs are host-rearranged so every DMA's src layout matches
    # its SBUF dst layout exactly (>=1.2 KB contiguous descriptors):
    #   enc_d [p, c, n] = enc.T[(c p), n], dec_d likewise
    #   wenc_d [dc, p, jc, d'] = W_enc[jc*128+p, dc*128+d']
    #   wdect_d [jc, p, c, j'] = W_dec.T[c*128+p, jc*128+j']
    #   wout_d [jc, p, v] = W_out.T[jc*128+p, v]
    enc_d = nc.dram_tensor("enc_d", [P, DC, BL * T], f32, kind="ExternalInput").ap()
    dec_d = nc.dram_tensor("dec_d", [P, DC, BL * U], f32, kind="ExternalInput").ap()
    wenc_d = nc.dram_tensor("wenc_d", [P, DC, DC, P], f32, kind="ExternalInput").ap()
    wdect_d = nc.dram_tensor("wdect_d", [P, DC, DC, P], f32, kind="ExternalInput").ap()
    wout_d = nc.dram_tensor("wout_d", [P, DC, V], f32, kind="ExternalInput").ap()
    aux_d = nc.dram_tensor("aux_d", [BL * U, AUXW], f32, kind="ExternalInput").ap()
    bias_d = nc.dram_tensor("bias_d", [P, 2 * DC], f32, kind="ExternalInput").ap()
    out = nc.dram_tensor("out", [BL, T, U, V], bf16, kind="ExternalOutput").ap()

    with tile.TileContext(nc) as tc:
        with ExitStack() as ctx:
            main = ctx.enter_context(tc.tile_pool(name="main", bufs=1))
            ps_a = ctx.enter_context(tc.tile_pool(name="ps_a", bufs=2, space="PSUM"))
            crep_ps = ctx.enter_context(
                tc.tile_pool(name="crep_ps", bufs=3, space="PSUM")
            )
            out_pool = ctx.enter_context(tc.tile_pool(name="outp", bufs=3))

            def body():
                # ---- persistent tiles ----
                enc_sb = main.tile([P, DC, BL * T], f32r, name="enc", tag="enc")
                dec_sb = main.tile([P, DC, BL * U], f32r, name="dec", tag="dec")
                wenc_sb = main.tile([P, DC, DC, P], f32r, name="wenc", tag="wenc")
                wdecT_sb = main.tile([P, DC, DC, P], f32r, name="wdecT", tag="wdecT")
                woutT_sb = main.tile([P, DC, V], f32r, name="woutT", tag="woutT")
                aux_sb = main.tile([BL * U, AUXW], f32r, name="aux", tag="aux")
                warm_sb = main.tile([1, 8], f32, name="warm", tag="warm")
                biasin_sb = main.tile([P, 2 * DC], f32, name="biasin", tag="biasin")
                bsum_sb = main.tile([P, DC], f32, name="bsum", tag="bsum")
                benc_sb = biasin_sb[:, 0:DC]
                bdec_sb = biasin_sb[:, DC : 2 * DC]
                eye_sb = aux_sb[0 : BL * U, 0 : BL * U]
                ones_sb = aux_sb[0:1, 64 : 64 + P]
                bout_sb = aux_sb[0:1, 192 : 192 + V]
                wceT_sb = [
                    main.tile([P, V], f32r, name=f"wceT{i}", tag=f"wceT{i}")
                    for i in range(DC)
                ]
                dpT_sb = [
                    main.tile([P, BL * U], f32r, name=f"dpT{i}", tag=f"dpT{i}")
                    for i in range(DC)
                ]
                a_sb = [
                    main.tile([P, V], bf16, name=f"a{i}", tag=f"a{i}")
                    for i in range(BL * len(T_CHUNKS))
                ]
                c_sb = main.tile([BL * U, V], f32r, name="c", tag="c")
                crepg = [
                    main.tile([P, UH, V], bf16, name=f"crepg{i}", tag=f"crepg{i}")
                    for i in range(4)
                ]

                # ---- input DMAs: C-path inputs first, big weights chunked so
                # the dpT/C/wce cascade starts as each 0.26 MB chunk lands ----
                # The ACT queue gets NO DMAs: dma_start occupies the issuing
                # sequencer for ~1us, which would stall the ACT copies that the
                # crep pipeline depends on. SP + DVE queues carry all DMAs.
                # 8 consolidated input DMAs (per-DMA issue overhead on HW is
                # ~1-3 us serialized on the queue, so fewer is faster); the
                # ACT queue takes three early ones (its first copy is at ~6us)
                # dummy Identity to trigger the ~1.5us ACT table load during
                # the input phase instead of before the first dpT copy
                nc.any.memset(warm_sb[:], 0.0)
                nc.scalar.activation(
                    warm_sb[:], warm_sb[:], mybir.ActivationFunctionType.Identity,
                    bias=0.0,
                )
                nc.sync.dma_start(aux_sb[:], r(aux_d[:]))
                nc.sync.dma_start(biasin_sb[:], bias_d[:])
                nc.sync.dma_start(dec_sb[:], r(dec_d[:]))
                nc.vector.tensor_add(bsum_sb[:], benc_sb[:], bdec_sb[:])
                for jc in range(DC):
                    nc.sync.dma_start(
                        wdecT_sb[:, jc, :, :], r(wdect_d[:, jc, :, :])
                    )
                    nc.scalar.dma_start(woutT_sb[:, jc, :], r(wout_d[:, jc, :]))
                nc.scalar.dma_start(wenc_sb[:], r(wenc_d[:]))
                nc.sync.dma_start(enc_sb[:, :, 0:T], r(enc_d[:, :, 0:T]))
                nc.sync.dma_start(enc_sb[:, :, T : BL * T], r(enc_d[:, :, T : BL * T]))

                # ---- dec_projT[jc][j, m] = sum_d W_dec[j,d] dec[m,d] + bsum[j]
                #      interleaved with C[m, v] = dec_projT.T @ W_out.T + b_out ----
                ps_c = crep_ps.tile([P, 2, 512], f32, name="cps", tag="cps")
                for jc in range(DC):
                    psd = crep_ps.tile([P, 2, 512], f32, name="cps", tag="cps")
                    ps = psd[:, 0, 0 : BL * U]
                    for dc in range(DC):
                        nc.tensor.matmul(
                            ps,
                            wdecT_sb[:, jc, dc, :],
                            dec_sb[:, dc, :],
                            start=(dc == 0),
                            stop=(dc == DC - 1),
                        )
                    nc.scalar.activation(
                        dpT_sb[jc][:],
                        ps,
                        mybir.ActivationFunctionType.Identity,
                        bias=bsum_sb[:, jc : jc + 1],
                    )
                    nc.tensor.matmul(
                        ps_c[0 : BL * U, 0, 0:V],
                        dpT_sb[jc][:],
                        woutT_sb[:, jc, :],
                        start=(jc == 0),
                        stop=False,
                    )
                nc.tensor.matmul(
                    ps_c[0 : BL * U, 0, 0:V],
                    ones_sb[0:1, 0 : BL * U],
                    bout_sb[:],
                    start=False,
                    stop=True,
                )
                nc.scalar.copy(c_sb[:], ps_c[0 : BL * U, 0, 0:V])

                # ---- Wce fusion (copies on DVE; ACT is busy with crep) ----
                def emit_wce_chunk(dc):
                    psw = crep_ps.tile([P, 2, 512], f32, name="cps", tag="cps")
                    ps = psw[:, 0, 0:V]
                    for jc in range(DC):
                        nc.tensor.matmul(
                            ps,
                            wenc_sb[:, dc, jc, :],
                            woutT_sb[:, jc, :],
                            start=(jc == 0),
                            stop=(jc == DC - 1),
                        )
                    nc.vector.tensor_copy(wceT_sb[dc][:], ps)

                def emit_a(bl, tci, interleaved=False, on_dve=False):
                    t0, tn = T_CHUNKS[tci]
                    n0 = bl * T + t0
                    ps = ps_a.tile([P, V], f32, name="psa", tag="a")
                    for dc in range(DC):
                        if interleaved:
                            emit_wce_chunk(dc)
                        nc.tensor.matmul(
                            ps[:tn, :],
                            enc_sb[:, dc, n0 : n0 + tn],
                            wceT_sb[dc][:],
                            start=(dc == 0),
                            stop=(dc == DC - 1),
                        )
                    dst = a_sb[bl * len(T_CHUNKS) + tci][:tn, :]
                    if on_dve:
                        nc.vector.tensor_copy(dst, ps[:tn, :])
                    else:
                        nc.scalar.copy(dst, ps[:tn, :])

                # ---- crep: broadcast C rows via identity-column matmuls ----
                def emit_crep(bl, g, lo=0, hi=UH):
                    i = lo
                    while i < hi:
                        row = bl * U + g * UH + i
                        n = min(2, hi - i)
                        cp = crep_ps.tile([P, 2, 512], f32, name="cps", tag="cps")
                        for k in range(n):
                            nc.tensor.matmul(
                                cp[:, k, 0:V],
                                eye_sb[:, row + k : row + k + 1].broadcast_to(
                                    [BL * U, P]
                                ),
                                c_sb[:],
                                start=True,
                                stop=True,
                            )
                        nc.scalar.copy(
                            crepg[bl * 2 + g][:, i : i + n, :], cp[:, :n, 0:V]
                        )
                        i += n

                # ---- output tiles: ot[t, u, v] = A[t,v] + crep[u][t,v] ----
                qi = [0]

                def emit_out_tile(bl, tci, g, sliced=False):
                    t0, tn = T_CHUNKS[tci]
                    a = a_sb[bl * len(T_CHUNKS) + tci]
                    cg = crepg[bl * 2 + g]
                    ot = out_pool.tile([P, UH, V], bf16, name="ot", tag="ot")
                    ab = a[:tn, :].unsqueeze(1)
                    slices = ((0, 5), (5, 5), (10, 5)) if sliced else ((0, UH),)
                    for s0, sn in slices:
                        nc.vector.tensor_add(
                            ot[:tn, s0 : s0 + sn, :],
                            ab.broadcast_to([tn, sn, V]),
                            cg[:tn, s0 : s0 + sn, :],
                        )
                        q = nc.sync
                        qi[0] += 1
                        q.dma_start(
                            out[bl, t0 : t0 + tn, g * UH + s0 : g * UH + s0 + sn, :],
                            ot[:tn, s0 : s0 + sn, :],
                        )

                emit_crep(0, 0, 0, 6)
                emit_a(0, 0, interleaved=True, on_dve=True)
                emit_crep(0, 0, 6, UH)
                emit_a(0, 1, on_dve=True)
                emit_out_tile(0, 0, 0, sliced=True)
                emit_a(0, 2)
                emit_crep(0, 1)
                emit_out_tile(0, 1, 0)
                emit_a(1, 0)
                emit_out_tile(0, 2, 0)
                emit_crep(1, 0)
                emit_a(1, 1)
                emit_a(1, 2)
                emit_out_tile(0, 0, 1)
                emit_crep(1, 1)
                emit_out_tile(0, 1, 1)
                emit_out_tile(0, 2, 1)
                emit_out_tile(1, 0, 0)
                emit_out_tile(1, 1, 0)
                emit_out_tile(1, 2, 0)
                emit_out_tile(1, 0, 1)
                emit_out_tile(1, 2, 1)
                emit_out_tile(1, 1, 1, sliced=True)

            if reps == 1:
                body()
            else:
                with tc.For_i(
                    0,
                    reps,
                    1,
                    hint_engines=(mybir.EngineType.PE, mybir.EngineType.Activation),
                ):
                    body()

    nc.compile()
    return nc


def _host_prep(inputs):
    """Per-core input maps. Layout-only host work (slice/transpose/reshape)."""
    enc = np.ascontiguousarray(inputs["encoder_out"], dtype=np.float32)
    dec = np.ascontiguousarray(inputs["decoder_out"], dtype=np.float32)
    # wenc_d[dc, p, jc, d'] = W_enc[jc*128+p, dc*128+d']
    wenc_d = np.ascontiguousarray(
        np.asarray(inputs["W_enc"], dtype=np.float32)
        .reshape(DC, P, DC, P)
        .transpose(1, 2, 0, 3)
    )
    # wdect_d[jc, p, c, j'] = W_dec.T[c*128+p, jc*128+j'] = W_dec[jc*128+j', c*128+p]
    wdect_d = np.ascontiguousarray(
        np.asarray(inputs["W_dec"], dtype=np.float32)
        .T.reshape(DC, P, DC, P)
        .transpose(1, 2, 0, 3)
    )
    # wout_d[jc, p, v] = W_out.T[jc*128+p, v] = W_out[v, jc*128+p]
    wout_d = np.ascontiguousarray(
        np.asarray(inputs["W_out"], dtype=np.float32).T.reshape(DC, P, V).transpose(1, 0, 2)
    )
    AUXW = 704
    aux = np.zeros((BL * U, AUXW), dtype=np.float32)
    aux[0 : BL * U, 0 : BL * U] = np.eye(BL * U, dtype=np.float32)
    aux[0:1, 64 : 64 + P] = 1.0
    aux[0:1, 192 : 192 + V] = inputs["b_out"].reshape(1, V)
    bias_np = np.concatenate(
        [
            inputs["b_enc"].reshape(DC, P).T,
            inputs["b_dec"].reshape(DC, P).T,
        ],
        axis=1,
    ).astype(np.float32)

    in_maps = []
    for c in range(NCORES):
        b0 = c * BL
        # enc_d[p, dc, n] = enc.T[dc*128+p, n]
        enc_c = np.ascontiguousarray(
            enc[b0 : b0 + BL].reshape(BL * T, DC, P).transpose(2, 1, 0)
        )
        dec_c = np.ascontiguousarray(
            dec[b0 : b0 + BL].reshape(BL * U, DC, P).transpose(2, 1, 0)
        )
        in_maps.append(
            {
                "enc_d": enc_c,
                "dec_d": dec_c,
                "wenc_d": wenc_d,
                "wdect_d": wdect_d,
                "wout_d": wout_d,
                "aux_d": aux,
                "bias_d": bias_np,
            }
        )
    return in_maps


def get_program(reps=1):
    key = f"nc{reps}"
    if key not in _CACHE:
        _CACHE[key] = _build_program(reps)
    return _CACHE[key]


def kernel(**inputs) -> np.ndarray:
    from concourse.bass_utils import run_bass_kernel_spmd

    nc = get_program()
    in_maps = _host_prep(inputs)
    res = run_bass_kernel_spmd(nc, in_maps, list(range(NCORES)))
    return np.concatenate(
        [np.asarray(r["out"]).astype(np.float32) for r in res.results], axis=0
    )


# revision 44
# speedup vs baseline: 1.1141x; 1.0127x over previous
"""RNN-T Joiner kernel for Trainium2 (Bass/Tile), SPMD over 8 NeuronCores.

Math: logits[b,t,u,v] = (enc@W_enc.T + b_enc + dec@W_dec.T + b_dec) @ W_out.T + b_out
    = A[b,t,v] + C[b,u,v]
where A = enc @ (W_out@W_enc).T  (no bias)
      C = (dec@W_dec.T + b_enc + b_dec) @ W_out.T + b_out

The (B,T,U,512)@(512,500) product in the reference (73.7 GFLOP) collapses by
linearity into two small matmuls plus a broadcast add, leaving the kernel
output-DMA / DVE-add bound (18 MB/core bf16 out at ~315 GB/s/core measured;
9M-element bf16 adds at DVE 2x_1p mode).

Design (per core, 2 batches) — iterated against real-HW NTFF traces:
- NO Pool/GPSIMD compute: on HW each GPSIMD op costs ~2 us (the one-hot
  'sel' broadcast pipeline of the original version measured 129 us
  standalone vs ~20 us modeled).
- C-row -> 128-partition broadcast via PE matmuls with a stride-0
  (broadcast) identity-column lhsT: crep[p,u,:] = eye[:,u](bcast 128).T @ C.
  HW-verified; reads C's SBUF tile directly, no staging, any row index.
  PSUM rows padded to 512 f32 (bank alignment), 2 rows per tile, 3 rotating
  tiles; PSUM->bf16 copies run on ACT while DVE does the adds.
- C path reassociated: dec_projT = (dec @ W_dec.T).T via 16 tiny matmuls
  (pipelined through the same 3-buf PSUM pool), b_enc+b_dec folded into the
  dec_projT copies as a per-partition ACT bias, C = dec_projT.T @ W_out.T
  + ones.T@b_out. Wce = W_out@W_enc fusion chunks rotate through the same
  pool; wce/a0/a1 PSUM->SBUF copies go to the (pre-add idle) DVE.
- The broadcast add A[t,v]+C[u,v] is one wide bf16 DVE tensor_add per
  (batch, t-chunk, u-group) (free 15*500, 2x_1p); on HW the 12 whole adds
  stream back-to-back at ~3.98 us. First and last groups are sliced in 5-u
  chunks to cut pipeline head/tail latency.
- Input DMAs: per-DMA end-to-end latency on HW is ~2.5 us serialized per
  queue, so small inputs are packed (aux: eye/ones/b_out; bias pair) and
  the three weight matrices load in host-pre-transposed, per-partition-
  contiguous layouts (~2 KB descriptors). W_dec.T/W_out.T load as per-jc
  chunks so the dec_projT/C cascade starts as each chunk lands; they are
  split across the SP and ACT queues. A dummy Identity op pre-triggers the
  ~1.5 us ACT table load during the input phase.
- Output: bf16 logits (rel err ~5e-3 vs f32 reference; host upcasts on
  gather), whole-group [tn,15,500] DMAs (15000 B contiguous DRAM rows),
  all on the SP queue so the ACT sequencer never stalls on DMA bookkeeping.

Sharding: data-parallel over batch B=16 -> 2 per core, no collectives.
Host-side work is layout only (slice / transpose / reshape / constants)
plus the final bf16->f32 upcast of the gathered output.
"""

import numpy as np

B, T, U, D, V = 16, 300, 30, 512, 500
NCORES = 8
BL = B // NCORES  # batches per core
P = 128
DC = D // P  # 4 contraction chunks

T_CHUNKS = [(0, 128), (128, 128), (256, 44)]
UH = 15  # u-group size (2 groups of 15)

_CACHE = {}


def _build_program(reps=1):
    from contextlib import ExitStack

    import concourse.bass as bass
    import concourse.tile as tile
    from concourse import bacc, mybir

    f32 = mybir.dt.float32
    f32r = mybir.dt.float32r
    bf16 = mybir.dt.bfloat16

    def r(ap):
        return ap.bitcast(f32r)

    nc = bacc.Bacc("TRN2", target_bir_lowering=False, debug=False)

    # aux packs the f32r matmul constants into one DMA -> one f32r tile
    # (fp32r operands must be real f32r tiles, not bitcast slices):
    # eye rows 0:60 cols 0:60, ones row 0 cols 64:192, bout row 0 cols 192:692.
    # bias_d packs b_enc/b_dec column-chunked: [128, 0:4]=benc, [:, 4:8]=bdec.
    AUXW = 704
    # weight/act inputs are host-rearranged so every DMA's src layout matches
    # its SBUF dst layout exactly (>=1.2 KB contiguous descriptors):
    #   enc_d [p, c, n] = enc.T[(c p), n], dec_d likewise
    #   wenc_d [dc, p, jc, d'] = W_enc[jc*128+p, dc*128+d']
    #   wdect_d [jc, p, c, j'] = W_dec.T[c*128+p, jc*128+j']
    #   wout_d [jc, p, v] = W_out.T[jc*128+p, v]
    enc_d = nc.dram_tensor("enc_d", [P, DC, BL * T], f32, kind="ExternalInput").ap()
    # bundleDW: dec chunks at cols dc*64 (width 60), W_dec.T at 256 + jc*512
    # + c*128 — the dpT/C-critical inputs in ONE 9 KB/partition-descriptor DMA
    bundledw_d = nc.dram_tensor("bundledw_d", [P, 2304], f32, kind="ExternalInput").ap()
    wenc_d = nc.dram_tensor("wenc_d", [P, DC, DC, P], f32, kind="ExternalInput").ap()
    wout_d = nc.dram_tensor("wout_d", [P, DC, V], f32, kind="ExternalInput").ap()
    aux_d = nc.dram_tensor("aux_d", [BL * U, AUXW], f32, kind="ExternalInput").ap()
    bias_d = nc.dram_tensor("bias_d", [P, 2 * DC], f32, kind="ExternalInput").ap()
    out = nc.dram_tensor("out", [BL, T, U, V], bf16, kind="ExternalOutput").ap()

    with tile.TileContext(nc) as tc:
        with ExitStack() as ctx:
            main = ctx.enter_context(tc.tile_pool(name="main", bufs=1))
            ps_a = ctx.enter_context(tc.tile_pool(name="ps_a", bufs=2, space="PSUM"))
            crep_ps = ctx.enter_context(
                tc.tile_pool(name="crep_ps", bufs=3, space="PSUM")
            )
            out_pool = ctx.enter_context(tc.tile_pool(name="outp", bufs=3))

            def body():
                # ---- persistent tiles ----
                enc_sb = main.tile([P, DC, BL * T], f32r, name="enc", tag="enc")
                bigDW = main.tile([P, 2304], f32r, name="bigDW", tag="bigDW")

                def dec_v(dc):
                    return bigDW[:, dc * 64 : dc * 64 + BL * U]

                def wdecT_v(jc, dc):
                    o = 256 + jc * 512 + dc * 128
                    return bigDW[:, o : o + 128]
                wenc_sb = main.tile([P, DC, DC, P], f32r, name="wenc", tag="wenc")
                woutT_sb = main.tile([P, DC, V], f32r, name="woutT", tag="woutT")
                aux_sb = main.tile([BL * U, AUXW], f32r, name="aux", tag="aux")
                warm_sb = main.tile([1, 8], f32, name="warm", tag="warm")
                biasin_sb = main.tile([P, 2 * DC], f32, name="biasin", tag="biasin")
                bsum_sb = main.tile([P, DC], f32, name="bsum", tag="bsum")
                benc_sb = biasin_sb[:, 0:DC]
                bdec_sb = biasin_sb[:, DC : 2 * DC]
                eye_sb = aux_sb[0 : BL * U, 0 : BL * U]
                ones_sb = aux_sb[0:1, 64 : 64 + P]
                bout_sb = aux_sb[0:1, 192 : 192 + V]
                wceT_sb = [
                    main.tile([P, V], f32r, name=f"wceT{i}", tag=f"wceT{i}")
                    for i in range(DC)
                ]
                dpT_sb = [
                    main.tile([P, BL * U], f32r, name=f"dpT{i}", tag=f"dpT{i}")
                    for i in range(DC)
                ]
                a_sb = [
                    main.tile([P, V], bf16, name=f"a{i}", tag=f"a{i}")
                    for i in range(BL * len(T_CHUNKS))
                ]
                c_sb = main.tile([BL * U, V], f32r, name="c", tag="c")
                crepg = [
                    main.tile([P, UH, V], bf16, name=f"crepg{i}", tag=f"crepg{i}")
                    for i in range(4)
                ]

                # ---- input DMAs: C-path inputs first, big weights chunked so
                # the dpT/C/wce cascade starts as each 0.26 MB chunk lands ----
                # The ACT queue gets NO DMAs: dma_start occupies the issuing
                # sequencer for ~1us, which would stall the ACT copies that the
                # crep pipeline depends on. SP + DVE queues carry all DMAs.
                # 8 consolidated input DMAs (per-DMA issue overhead on HW is
                # ~1-3 us serialized on the queue, so fewer is faster); the
                # ACT queue takes three early ones (its first copy is at ~6us)
                # dummy Identity to trigger the ~1.5us ACT table load during
                # the input phase instead of before the first dpT copy
                nc.any.memset(warm_sb[:], 0.0)
                nc.scalar.activation(
                    warm_sb[:], warm_sb[:], mybir.ActivationFunctionType.Identity,
                    bias=0.0,
                )
                nc.sync.dma_start(bigDW[:], r(bundledw_d[:]))
                nc.sync.dma_start(biasin_sb[:], bias_d[:])
                nc.sync.dma_start(aux_sb[:], r(aux_d[:]))
                nc.vector.tensor_add(bsum_sb[:], benc_sb[:], bdec_sb[:])
                for jc in range(DC):
                    nc.scalar.dma_start(woutT_sb[:, jc, :], r(wout_d[:, jc, :]))
                nc.scalar.dma_start(wenc_sb[:], r(wenc_d[:]))
                nc.sync.dma_start(enc_sb[:, :, 0:T], r(enc_d[:, :, 0:T]))
                nc.sync.dma_start(enc_sb[:, :, T : BL * T], r(enc_d[:, :, T : BL * T]))

                # ---- dec_projT[jc][j, m] = sum_d W_dec[j,d] dec[m,d] + bsum[j]
                #      interleaved with C[m, v] = dec_projT.T @ W_out.T + b_out ----
                ps_c = crep_ps.tile([P, 2, 512], f32, name="cps", tag="cps")
                for jc in range(DC):
                    psd = crep_ps.tile([P, 2, 512], f32, name="cps", tag="cps")
                    ps = psd[:, 0, 0 : BL * U]
                    for dc in range(DC):
                        nc.tensor.matmul(
                            ps,
                            wdecT_v(jc, dc),
                            dec_v(dc),
                            start=(dc == 0),
                            stop=(dc == DC - 1),
                        )
                    nc.scalar.activation(
                        dpT_sb[jc][:],
                        ps,
                        mybir.ActivationFunctionType.Identity,
                        bias=bsum_sb[:, jc : jc + 1],
                    )
                    nc.tensor.matmul(
                        ps_c[0 : BL * U, 0, 0:V],
                        dpT_sb[jc][:],
                        woutT_sb[:, jc, :],
                        start=(jc == 0),
                        stop=False,
                    )
                nc.tensor.matmul(
                    ps_c[0 : BL * U, 0, 0:V],
                    ones_sb[0:1, 0 : BL * U],
                    bout_sb[:],
                    start=False,
                    stop=True,
                )
                nc.scalar.copy(c_sb[:], ps_c[0 : BL * U, 0, 0:V])

                # ---- Wce fusion (copies on DVE; ACT is busy with crep) ----
                def emit_wce_chunk(dc):
                    psw = crep_ps.tile([P, 2, 512], f32, name="cps", tag="cps")
                    ps = psw[:, 0, 0:V]
                    for jc in range(DC):
                        nc.tensor.matmul(
                            ps,
                            wenc_sb[:, dc, jc, :],
                            woutT_sb[:, jc, :],
                            start=(jc == 0),
                            stop=(jc == DC - 1),
                        )
                    nc.vector.tensor_copy(wceT_sb[dc][:], ps)

                def emit_a(bl, tci, interleaved=False, on_dve=False):
                    t0, tn = T_CHUNKS[tci]
                    n0 = bl * T + t0
                    ps = ps_a.tile([P, V], f32, name="psa", tag="a")
                    for dc in range(DC):
                        if interleaved:
                            emit_wce_chunk(dc)
                        nc.tensor.matmul(
                            ps[:tn, :],
                            enc_sb[:, dc, n0 : n0 + tn],
                            wceT_sb[dc][:],
                            start=(dc == 0),
                            stop=(dc == DC - 1),
                        )
                    dst = a_sb[bl * len(T_CHUNKS) + tci][:tn, :]
                    if on_dve:
                        nc.vector.tensor_copy(dst, ps[:tn, :])
                    else:
                        nc.scalar.copy(dst, ps[:tn, :])

                # ---- crep: broadcast C rows via identity-column matmuls ----
                def emit_crep(bl, g, lo=0, hi=UH):
                    i = lo
                    while i < hi:
                        row = bl * U + g * UH + i
                        n = min(2, hi - i)
                        cp = crep_ps.tile([P, 2, 512], f32, name="cps", tag="cps")
                        for k in range(n):
                            nc.tensor.matmul(
                                cp[:, k, 0:V],
                                eye_sb[:, row + k : row + k + 1].broadcast_to(
                                    [BL * U, P]
                                ),
                                c_sb[:],
                                start=True,
                                stop=True,
                            )
                        nc.scalar.copy(
                            crepg[bl * 2 + g][:, i : i + n, :], cp[:, :n, 0:V]
                        )
                        i += n

                # ---- output tiles: ot[t, u, v] = A[t,v] + crep[u][t,v] ----
                qi = [0]

                def emit_out_tile(bl, tci, g, sliced=False):
                    t0, tn = T_CHUNKS[tci]
                    a = a_sb[bl * len(T_CHUNKS) + tci]
                    cg = crepg[bl * 2 + g]
                    ot = out_pool.tile([P, UH, V], bf16, name="ot", tag="ot")
                    ab = a[:tn, :].unsqueeze(1)
                    slices = ((0, 5), (5, 5), (10, 5)) if sliced else ((0, UH),)
                    for s0, sn in slices:
                        nc.vector.tensor_add(
                            ot[:tn, s0 : s0 + sn, :],
                            ab.broadcast_to([tn, sn, V]),
                            cg[:tn, s0 : s0 + sn, :],
                        )
                        q = nc.sync
                        qi[0] += 1
                        q.dma_start(
                            out[bl, t0 : t0 + tn, g * UH + s0 : g * UH + s0 + sn, :],
                            ot[:tn, s0 : s0 + sn, :],
                        )

                emit_crep(0, 0, 0, 6)
                emit_a(0, 0, interleaved=True, on_dve=True)
                emit_crep(0, 0, 6, UH)
                emit_a(0, 1, on_dve=True)
                emit_out_tile(0, 0, 0, sliced=True)
                emit_a(0, 2)
                emit_crep(0, 1)
                emit_out_tile(0, 1, 0)
                emit_a(1, 0)
                emit_out_tile(0, 2, 0)
                emit_crep(1, 0)
                emit_a(1, 1)
                emit_a(1, 2)
                emit_out_tile(0, 0, 1)
                emit_crep(1, 1)
                emit_out_tile(0, 1, 1)
                emit_out_tile(0, 2, 1)
                emit_out_tile(1, 0, 0)
                emit_out_tile(1, 1, 0)
                emit_out_tile(1, 2, 0)
                emit_out_tile(1, 0, 1)
                emit_out_tile(1, 2, 1)
                emit_out_tile(1, 1, 1, sliced=True)

            if reps == 1:
                body()
            else:
                with tc.For_i(
                    0,
                    reps,
                    1,
                    hint_engines=(mybir.EngineType.PE, mybir.EngineType.Activation),
                ):
                    body()

    nc.compile()
    return nc


def _host_prep(inputs):
    """Per-core input maps. Layout-only host work (slice/transpose/reshape)."""
    enc = np.ascontiguousarray(inputs["encoder_out"], dtype=np.float32)
    dec = np.ascontiguousarray(inputs["decoder_out"], dtype=np.float32)
    # wenc_d[dc, p, jc, d'] = W_enc[jc*128+p, dc*128+d']
    wenc_d = np.ascontiguousarray(
        np.asarray(inputs["W_enc"], dtype=np.float32)
        .reshape(DC, P, DC, P)
        .transpose(1, 2, 0, 3)
    )
    # wdect_h[p, jc, c, j'] = W_dec.T[c*128+p, jc*128+j']
    wdect_h = (
        np.asarray(inputs["W_dec"], dtype=np.float32)
        .T.reshape(DC, P, DC, P)
        .transpose(1, 2, 0, 3)
        .reshape(P, 2048)
    )
    # wout_d[jc, p, v] = W_out.T[jc*128+p, v] = W_out[v, jc*128+p]
    wout_d = np.ascontiguousarray(
        np.asarray(inputs["W_out"], dtype=np.float32).T.reshape(DC, P, V).transpose(1, 0, 2)
    )
    AUXW = 704
    aux = np.zeros((BL * U, AUXW), dtype=np.float32)
    aux[0 : BL * U, 0 : BL * U] = np.eye(BL * U, dtype=np.float32)
    aux[0:1, 64 : 64 + P] = 1.0
    aux[0:1, 192 : 192 + V] = inputs["b_out"].reshape(1, V)
    bias_np = np.concatenate(
        [
            inputs["b_enc"].reshape(DC, P).T,
            inputs["b_dec"].reshape(DC, P).T,
        ],
        axis=1,
    ).astype(np.float32)

    in_maps = []
    for c in range(NCORES):
        b0 = c * BL
        # enc_d[p, dc, n] = enc.T[dc*128+p, n]
        enc_c = np.ascontiguousarray(
            enc[b0 : b0 + BL].reshape(BL * T, DC, P).transpose(2, 1, 0)
        )
        dec_c = dec[b0 : b0 + BL].reshape(BL * U, DC, P).transpose(2, 1, 0)
        bdw = np.zeros((P, 2304), dtype=np.float32)
        for dc in range(DC):
            bdw[:, dc * 64 : dc * 64 + BL * U] = dec_c[:, dc, :]
        bdw[:, 256:2304] = wdect_h
        in_maps.append(
            {
                "enc_d": enc_c,
                "bundledw_d": bdw,
                "wenc_d": wenc_d,
                "wout_d": wout_d,
                "aux_d": aux,
                "bias_d": bias_np,
            }
        )
    return in_maps


def get_program(reps=1):
    key = f"nc{reps}"
    if key not in _CACHE:
        _CACHE[key] = _build_program(reps)
    return _CACHE[key]


def kernel(**inputs) -> np.ndarray:
    from concourse.bass_utils import run_bass_kernel_spmd

    nc = get_program()
    in_maps = _host_prep(inputs)
    res = run_bass_kernel_spmd(nc, in_maps, list(range(NCORES)))
    return np.concatenate(
        [np.asarray(r["out"]).astype(np.float32) for r in res.results], axis=0
    )
